# revision 42
# baseline (speedup 1.0000x reference)
"""Trainium2 Bass kernel for nn_Disc_edge2 (3-layer dense-graph GNN + MLP head).

Sharding: data-parallel over batch B=16 across 8 cores (2 graphs/core).
Per-graph on-chip layout: msg tensors are [d=128 partitions, f=16384 free]
with free index f = c1*2048 + t*128 + p  where the edge (i, j) maps to
p = i (inner 128) and j = 8*t + c1 (c1 = j%8, t = j//8).

This layout makes both per-node broadcast adds pure matmuls with CONSTANT
moving operands:
  - xi[i,:] broadcast over j  -> rhs = tiled identity [I I I I] (i = p inner)
  - xj[j,:] broadcast over i  -> rhs = SELJM[j', f] = (j' == 8t+c1), built
    once with one affine_select.
The adjacency mask is a replicated tensor built with a 0-step broadcast DMA
and applied with one bf16 tensor_tensor per chunk (scalar_tensor_tensor with
accum_out on layer 2, where the per-chunk sums become the readout mean).
The residual blends are plain adds; their 0.5 factors are folded into the
layer-2 weights.
"""

import os
import sys

sys.path.insert(0, "/opt/trn_rl_repo")

import numpy as np

import concourse.bass as bass
from concourse import bacc
import concourse.mybir as mybir
import concourse.tile as tile
from concourse.masks import make_identity

F32 = mybir.dt.float32
BF16 = mybir.dt.bfloat16
I32 = mybir.dt.int32
AF = mybir.ActivationFunctionType
OP = mybir.AluOpType

B, N, DN0, DE0, DH = 16, 128, 64, 16, 128
NCORES = 8
GPC = B // NCORES          # graphs per core
FREE = N * N               # 16384
CH = 512                   # columns per PSUM chunk
NCH = FREE // CH           # 32 chunks

WEIGHT_NAMES = [
    "w_msg_0", "b_msg_0", "w_node_0", "b_node_0",
    "w_msg_1", "b_msg_1", "w_node_1", "b_node_1",
    "w_msg_2", "b_msg_2", "w_node_2", "b_node_2",
    "w_h1", "b_h1", "w_h2", "b_h2", "w_h3", "b_h3",
]

_CACHE = {}


def build_nc():
    nc = bacc.Bacc()

    ei_d = nc.declare_dram_parameter("edge_index", [GPC, N, N], I32, isOutput=False)
    x_d = nc.declare_dram_parameter("x", [GPC, N, DN0], F32, isOutput=False)
    ea_d = nc.declare_dram_parameter("edge_attr", [GPC, N, N, DE0], F32, isOutput=False)
    wd = {}
    shapes = {
        "w_msg_0": [2 * DN0 + DE0, DH], "b_msg_0": [DH],
        "w_node_0": [DN0 + DH, DH], "b_node_0": [DH],
        "w_msg_1": [3 * DH, DH], "b_msg_1": [DH],
        "w_node_1": [2 * DH, DH], "b_node_1": [DH],
        "w_msg_2": [3 * DH, DH], "b_msg_2": [DH],
        "w_node_2": [2 * DH, DH], "b_node_2": [DH],
        "w_h1": [DH, DH], "b_h1": [DH],
        "w_h2": [DH, DH], "b_h2": [DH],
        "w_h3": [DH, 1], "b_h3": [1],
    }
    for n_ in WEIGHT_NAMES:
        wd[n_] = nc.declare_dram_parameter(n_, shapes[n_], F32, isOutput=False)
    out_d = nc.declare_dram_parameter("out", [GPC, 1], F32, isOutput=True)

    with tile.TileContext(nc) as tc:
        import contextlib
        stack = contextlib.ExitStack()
        consts = stack.enter_context(tc.tile_pool(name="consts", bufs=1))
        gbuf = stack.enter_context(tc.tile_pool(name="gbuf", bufs=1))
        small = stack.enter_context(tc.tile_pool(name="small", bufs=2))
        zpool = stack.enter_context(tc.tile_pool(name="zp", bufs=5, space="PSUM"))
        spsum = stack.enter_context(tc.tile_pool(name="sp", bufs=2, space="PSUM"))
        dpool = stack.enter_context(tc.tile_pool(name="dp", bufs=1, space="DRAM"))

        # -------- input loads first: head of the sync HWDGE FIFO --------
        e0nat, x0in, aiin = [], [], []
        for g in range(GPC):
            t = gbuf.tile([128, 128], I32, tag="ai")
            nc.sync.dma_start(t[:], ei_d[g])
            aiin.append(t)
            t = gbuf.tile([128, DN0], F32, tag=f"x0_{g}")
            nc.sync.dma_start(t[:], x_d[g])
            x0in.append(t)
        for g in range(GPC):
            t = gbuf.tile([128, N * DE0], F32, tag="e0nat")
            nc.sync.dma_start(t[:], ea_d[g].rearrange("i j k -> i (j k)"))
            e0nat.append(t)

        # -------- constants / weights (scalar-queue DMAs) --------
        def f2b(src_ap, p, name, scale=None):
            tmp = consts.tile([p, 128], F32, tag=f"tmp_{name}")
            nc.scalar.dma_start(tmp[:], src_ap)
            t = consts.tile([p, 128], BF16, tag=name)
            if scale is None:
                nc.vector.tensor_copy(t[:], tmp[:])
            else:
                nc.vector.tensor_scalar_mul(t[:], tmp[:], scale)
            return t

        ident = consts.tile([128, 128], F32, tag="ident")
        make_identity(nc, ident[:])

        w = {}
        w["Wi0"] = f2b(wd["w_msg_0"][0:DN0, :], DN0, "Wi0")
        w["Wj0"] = f2b(wd["w_msg_0"][DN0:2 * DN0, :], DN0, "Wj0")
        # 8 block variants of We0: rows [c1*16, c1*16+16) = We0, rest zero, so
        # the K=128 contraction against E0T[(cc,k), (t,p)] picks out cc == c1.
        we0b16 = f2b(wd["w_msg_0"][2 * DN0:, :], DE0, "we0b16")
        we0blk = []
        for c1 in range(8):
            blk = consts.tile([128, DH], BF16, tag=f"we0b{c1}")
            nc.vector.memset(blk[:], 0.0)
            nc.scalar.dma_start(blk[c1 * 16:(c1 + 1) * 16, :], we0b16[:])
            we0blk.append(blk)

        # tiled identity [I I I I] bf16: seli[p', (u,p)] = (p'==p)
        seli = consts.tile([128, CH], BF16, tag="seli")
        nc.gpsimd.memset(seli[:], 0.0)
        nc.gpsimd.affine_select(
            out=seli[:], in_=seli[:], compare_op=OP.not_equal, fill=1.0,
            base=0, pattern=[[0, 4], [-1, 128]], channel_multiplier=1)

        # SELJM[j', (c1,t,p)] = (j' == 8t + c1)
        seljm = consts.tile([128, FREE], BF16, tag="seljm")
        nc.gpsimd.memset(seljm[:], 0.0)
        nc.gpsimd.affine_select(
            out=seljm[:], in_=seljm[:], compare_op=OP.not_equal, fill=1.0,
            base=0, pattern=[[-1, 8], [-8, 16], [0, 128]], channel_multiplier=1)

        w["Wx0"] = f2b(wd["w_node_0"][0:DN0, :], DN0, "Wx0")
        w["Wa0"] = f2b(wd["w_node_0"][DN0:, :], DH, "Wa0")
        for l in (1, 2):
            sc = 0.5 if l == 2 else None
            w[f"Wi{l}"] = f2b(wd[f"w_msg_{l}"][0:DH, :], DH, f"Wi{l}", sc)
            w[f"Wj{l}"] = f2b(wd[f"w_msg_{l}"][DH:2 * DH, :], DH, f"Wj{l}", sc)
            w[f"We{l}"] = f2b(wd[f"w_msg_{l}"][2 * DH:, :], DH, f"We{l}", sc)
        w["Wx1"] = f2b(wd["w_node_1"][0:DH, :], DH, "Wx1")
        w["Wa1"] = f2b(wd["w_node_1"][DH:, :], DH, "Wa1")
        # head weights stay f32
        wh1 = consts.tile([DH, DH], F32, tag="wh1")
        nc.scalar.dma_start(wh1[:], wd["w_h1"][:, :])
        wh2 = consts.tile([DH, DH], F32, tag="wh2")
        nc.scalar.dma_start(wh2[:], wd["w_h2"][:, :])
        wh3 = consts.tile([DH, 1], F32, tag="wh3")
        nc.scalar.dma_start(wh3[:], wd["w_h3"][:, :])

        # bias rows replicated across partitions via 0-step broadcast DMA
        brep = {}
        for l in range(3):
            rep = consts.tile([128, DH], F32, tag=f"brep{l}")
            nc.scalar.dma_start(
                rep[:], wd[f"b_msg_{l}"][:].unsqueeze(0).to_broadcast([128, DH]))
            brep[l] = rep
        bcol = {}
        for nm in ("b_node_0", "b_node_1", "b_h1", "b_h2"):
            c = consts.tile([DH, 1], F32, tag=f"col_{nm}")
            nc.scalar.dma_start(c[:], wd[nm][:].unsqueeze(1))
            bcol[nm] = c
        bh3 = consts.tile([1, 1], F32, tag="col_bh3")
        nc.scalar.dma_start(bh3[:], wd["b_h3"][:].unsqueeze(1))

        # ---------------- per-graph pipeline ----------------
        for g in range(GPC):
            # ---- adjacency -> replicated mask RA ----
            af = gbuf.tile([128, 128], F32, tag="af")
            nc.vector.tensor_copy(af[:], aiin[g][:])      # int32 -> f32
            atp = spsum.tile([128, 128], F32, tag="sp")
            nc.tensor.transpose(atp[:], af[:], ident[:])  # AT[j,i] in PSUM
            atb = gbuf.tile([128, 128], BF16, tag="atb")
            nc.scalar.copy(atb[:], atp[:])
            atd = dpool.tile([128, 128], BF16, tag=f"atd{g}")
            nc.scalar.dma_start(atd[:], atb[:])
            # reorder AT[j,i] into mask-flat order f = (c1, t, p) in DRAM
            mfd = dpool.tile([FREE], BF16, tag=f"mfd{g}")
            nc.scalar.dma_start(
                mfd[:].rearrange("(c t p) -> c t p", c=8, t=16),
                atd[:].rearrange("(t c) p -> c t p", c=8))
            # replicate to all 128 partitions with a 0-step broadcast DMA
            ra = gbuf.tile([128, FREE], BF16, tag=f"ra{g}")
            nc.scalar.dma_start(ra[:], mfd[:].unsqueeze(0).to_broadcast([128, FREE]))

            # ---- e0 transpose: E0T[(c1,k), (t,p)] bf16 ----
            e0t = gbuf.tile([128, N * DE0], BF16, tag="e0t")
            for q in range(4):
                tp = zpool.tile([128, CH], F32, tag="z")
                for r in range(4):
                    t16 = 4 * q + r
                    nc.tensor.transpose(
                        tp[:, r * 128:(r + 1) * 128],
                        e0nat[g][:, 128 * t16:128 * (t16 + 1)], ident[:])
                nc.scalar.copy(e0t[:, q * CH:(q + 1) * CH], tp[:])

            # ---- x0T [c,i] bf16 ----
            x0tp = spsum.tile([128, 128], F32, tag="sp")
            nc.tensor.transpose(x0tp[0:DN0, :], x0in[g][:], ident[:])
            x0T = gbuf.tile([DN0, 128], BF16, tag="x0T")
            nc.scalar.copy(x0T[:], x0tp[0:DN0, :])

            msg0 = gbuf.tile([128, FREE], BF16, tag="msg0")
            msg1 = gbuf.tile([128, FREE], BF16, tag="msg1")
            bufA, bufB = (msg0, msg1) if g % 2 == 0 else (msg1, msg0)
            scratch = gbuf.tile([128, FREE // 4], BF16, tag="scratch")
            hsum = gbuf.tile([128, NCH], F32, tag="hsum")

            xT = x0T
            for layer in range(3):
                Wi, Wj = w[f"Wi{layer}"], w[f"Wj{layer}"]
                We = None if layer == 0 else w[f"We{layer}"]
                # xi' = xT.T @ Wi + b ; xj' = xT.T @ Wj   ([i,d] / [j,d])
                xip = spsum.tile([128, 128], F32, tag="sp")
                nc.tensor.matmul(xip[:], xT[:], Wi[:], start=True, stop=True)
                xib = small.tile([128, 128], BF16, tag="xib")
                nc.vector.tensor_add(xib[:], xip[:], brep[layer][:])
                xjp = spsum.tile([128, 128], F32, tag="sp")
                nc.tensor.matmul(xjp[:], xT[:], Wj[:], start=True, stop=True)
                xjb = small.tile([128, 128], BF16, tag="xjb")
                nc.scalar.copy(xjb[:], xjp[:])

                rhs_e = bufA if layer else None
                dst = bufA if layer == 0 else (bufB if layer == 1 else None)
                q4 = FREE // 4

                for k in range(NCH):
                    cols = slice(k * CH, (k + 1) * CH)
                    z = zpool.tile([128, CH], F32, tag="z")
                    if layer == 0:
                        c1, t4 = divmod(k, 4)
                        nc.tensor.matmul(
                            z[:], we0blk[c1][:],
                            e0t[:, t4 * CH:(t4 + 1) * CH],
                            start=True, stop=False)
                        cols = slice(c1 * 2048 + t4 * CH, c1 * 2048 + (t4 + 1) * CH)
                    else:
                        nc.tensor.matmul(z[:], We[:], rhs_e[:, cols],
                                         start=True, stop=False)
                    nc.tensor.matmul(z[:], xib[:], seli[:], start=False, stop=False)
                    nc.tensor.matmul(z[:], xjb[:], seljm[:, cols],
                                     start=False, stop=True)
                    # relu-evict on ACT, mask on DVE (accum_out = readout on L2)
                    if layer == 2:
                        # relu+mask+readout-accum in one in-place PSUM op;
                        # msg2 itself is never materialized
                        nc.vector.scalar_tensor_tensor(
                            out=z[:], in0=z[:], scalar=0.0, in1=ra[:, cols],
                            op0=OP.max, op1=OP.mult, accum_out=hsum[:, k:k + 1])
                    else:
                        raw = dst[:, cols]
                        nc.scalar.activation(raw, z[:], AF.Relu)
                        nc.vector.tensor_mul(raw, raw, ra[:, cols])

                if layer < 2:
                    # agg tree over (c1,t): 16384 -> 128, then node update
                    src = dst
                    nc.vector.tensor_add(scratch[:, 0:q4], src[:, 0:q4],
                                         src[:, q4:2 * q4])
                    nc.vector.tensor_add(scratch[:, 0:q4], scratch[:, 0:q4],
                                         src[:, 2 * q4:3 * q4])
                    nc.vector.tensor_add(scratch[:, 0:q4], scratch[:, 0:q4],
                                         src[:, 3 * q4:4 * q4])
                    width = q4
                    while width > 128:
                        h = width // 2
                        nc.vector.tensor_add(scratch[:, 0:h], scratch[:, 0:h],
                                             scratch[:, h:width])
                        width = h
                    aggT = small.tile([128, 128], BF16, tag="aggT")
                    nc.vector.tensor_copy(aggT[:], scratch[:, 0:128])

                    Wx, Wa = w[f"Wx{layer}"], w[f"Wa{layer}"]
                    xnp = spsum.tile([128, 128], F32, tag="sp")
                    nc.tensor.matmul(xnp[:], Wx[:], xT[:], start=True, stop=False)
                    nc.tensor.matmul(xnp[:], Wa[:], aggT[:], start=False, stop=True)
                    xnT = small.tile([128, 128], BF16, tag="xnT")
                    nc.scalar.activation(xnT[:], xnp[:], AF.Relu,
                                         bias=bcol[f"b_node_{layer}"][:])
                    if layer == 1:
                        # x-residual (x1+x2); the 0.5 is folded into Wi2/Wj2
                        xbl = small.tile([128, 128], BF16, tag="xbl")
                        nc.vector.tensor_add(xbl[:], xnT[:], xT[:])
                        xT = xbl
                    else:
                        xT = xnT

                if layer == 1:
                    # e-blend (bufA+bufB) -> bufA; the 0.5 is folded into We2
                    for k in range(NCH):
                        cols = slice(k * CH, (k + 1) * CH)
                        nc.vector.tensor_add(bufA[:, cols], bufA[:, cols],
                                             bufB[:, cols])

            # ---- readout head ----
            hpre = small.tile([128, 1], F32, tag="hpre")
            nc.vector.tensor_reduce(hpre[:], hsum[:], axis=mybir.AxisListType.X,
                                    op=OP.add)
            h1p = spsum.tile([128, 128], F32, tag="sp")
            nc.tensor.matmul(h1p[:, 0:1], wh1[:], hpre[:], start=True, stop=True)
            h1 = small.tile([128, 1], F32, tag="h1")
            nc.scalar.activation(h1[:], h1p[:, 0:1], AF.Relu,
                                 bias=bcol["b_h1"][:], scale=1.0 / FREE)
            h2p = spsum.tile([128, 128], F32, tag="sp")
            nc.tensor.matmul(h2p[:, 0:1], wh2[:], h1[:], start=True, stop=True)
            h2 = small.tile([128, 1], F32, tag="h2")
            nc.scalar.activation(h2[:], h2p[:, 0:1], AF.Relu, bias=bcol["b_h2"][:])
            h3p = spsum.tile([128, 128], F32, tag="sp")
            nc.tensor.matmul(h3p[0:1, 0:1], wh3[:], h2[:], start=True, stop=True)
            oval = small.tile([1, 1], F32, tag="oval")
            nc.scalar.activation(oval[:], h3p[0:1, 0:1], AF.Identity, bias=bh3[:])
            nc.sync.dma_start(out_d[g:g + 1, :], oval[:])

        stack.close()
    nc.finalize()
    return nc


def kernel(**inputs):
    inputs = {k: np.asarray(v) for k, v in inputs.items()}
    if "nc" not in _CACHE:
        _CACHE["nc"] = build_nc()
    nc = _CACHE["nc"]

    in_maps = []
    for c in range(NCORES):
        m = {
            "edge_index": np.ascontiguousarray(inputs["edge_index"][c * GPC:(c + 1) * GPC]),
            "x": np.ascontiguousarray(inputs["x"][c * GPC:(c + 1) * GPC]),
            "edge_attr": np.ascontiguousarray(inputs["edge_attr"][c * GPC:(c + 1) * GPC]),
        }
        for n_ in WEIGHT_NAMES:
            m[n_] = np.ascontiguousarray(inputs[n_], dtype=np.float32)
        in_maps.append(m)

    from concourse.bass_utils import run_bass_kernel_spmd
    res = run_bass_kernel_spmd(nc, in_maps, list(range(NCORES)))
    out = np.concatenate([np.asarray(res.results[c]["out"]).reshape(-1)
                          for c in range(NCORES)])
    return out.astype(np.float32)
